# revision 5
# baseline (speedup 1.0000x reference)
"""Gemma4 MoE feed-forward on 8 Trainium2 NeuronCores.

Strategy: expert-parallel. E == n_cores == 8, so core e owns expert e's
weights (Wg[e], Wu[e], Wd[e]) and receives exactly the tokens routed to
expert e (gathered + transposed + padded on the host). Each core runs a
dense gated-FFN over its token batch:

    dT = Wd^T @ (gelu_tanh(Wg^T x^T) * (Wu^T x^T))        (all [*, C] layouts)

The host then scatter-adds routing_weight * dT^T back into the full
[T, H] output. Tokens that select the same expert in both slots are
deduplicated on the host (weights summed).

All matmul operands are bf16 (rel err ~4e-3 vs the 2e-2 gate): bf16
streams the moving operand ~20ns/MM faster than fp32r and halves HBM
traffic. The up phase runs i-outer / n-inner so each Wg/Wu tile is
DMA'd exactly once (the old n-outer order re-streamed all up weights
per token block and saturated the 358 GB/s per-core HBM budget).
"""

import os
import sys

import numpy as np

for _p in ("/opt/trn_rl_repo", "/root/.axon_site/_ro/trn_rl_repo"):
    if os.path.isdir(_p) and _p not in sys.path:
        sys.path.append(_p)

T, H, I, E, K = 4096, 2048, 1024, 8, 2
NCORES = 8
KH = H // 128  # 16 k-tiles over the hidden dim
KI = I // 128  # 8 k-tiles over the intermediate dim
G = 8  # k-tiles per weight DMA (2KB/partition bf16 -> full per-queue DMA rate)

_PROGRAM_CACHE = {}
LAST_RESULT = None  # BassKernelResults of the most recent run (for test.py)
TRACE = False  # test.py sets this to capture an NTFF profile
TRACE_CORES = [0]

NWARM = int(os.environ.get("MOE_NWARM", "12"))


def _tile_w_up(W):
    """[H, I] -> [KI, GU, 128, G*128]: tile (k,i) of W at [i, k//G, :, (k%G)*128:],
    so each (i, g) DMA reads G*128*2 = 2KB contiguous per partition."""
    Wt = W.reshape(KH // G, G, 128, KI, 128).transpose(3, 0, 2, 1, 4)
    return np.ascontiguousarray(Wt).reshape(KI, KH // G, 128, G * 128)


def _tile_w_down(W):
    """[I, H] -> [KH, GD, 128, G*128] (same scheme, contraction over I)."""
    Wt = W.reshape(KI // G, G, 128, KH, 128).transpose(3, 0, 2, 1, 4)
    return np.ascontiguousarray(Wt).reshape(KH, KI // G, 128, G * 128)


def _pick_config(max_count):
    """Minimal uniform token-block config: NT blocks of even width N with
    NT*N >= max_count, N <= 512 (PSUM bank limit) and N >= 256 (so the
    ~100ns LDWEIGHTS fully hides under the moving-operand stream)."""
    mc = max(max_count, 256)
    nt = -(-mc // 512)
    n = -(-mc // nt)
    n += n % 2
    return (nt * n, nt, n)  # (C, NT, N)


def _build_program(C, NT, N):
    import concourse.tile as tile
    from concourse import bacc, mybir
    from contextlib import ExitStack

    GU = KH // G  # weight-DMA groups per i-tile (up phase)
    GD = KI // G  # weight-DMA groups per h-tile (down phase)

    f32 = mybir.dt.float32
    bf16 = mybir.dt.bfloat16

    nc = bacc.Bacc("TRN2", target_bir_lowering=False, debug=False)

    # x arrives host-packed in k-tile pairs: element [n, j, p, kk*N + c]
    # = x^T[(2j+kk)*128 + p, n*N + c], so each (n, j) DMA reads 2N*2 =
    # ~2KB contiguous per partition.
    xP = nc.dram_tensor("xP", [NT, KH // 2, 128, 2 * N], bf16, kind="ExternalInput").ap()
    Wg_d = nc.dram_tensor("Wg", [KI, GU, 128, G * 128], bf16, kind="ExternalInput").ap()
    Wu_d = nc.dram_tensor("Wu", [KI, GU, 128, G * 128], bf16, kind="ExternalInput").ap()
    Wd_d = nc.dram_tensor("Wd", [KH, GD, 128, G * 128], bf16, kind="ExternalInput").ap()
    dT = nc.dram_tensor("dT", [H, C], f32, kind="ExternalOutput").ap()

    # Partition-major view: row a*128+p -> partition p, free index a.
    dT_p = dT.rearrange("(a p) c -> p a c", p=128)  # [128, KH, C]

    GELU = mybir.ActivationFunctionType.Gelu_apprx_tanh

    with tile.TileContext(nc) as tc, ExitStack() as ctx:
        xpool = ctx.enter_context(tc.tile_pool(name="x", bufs=1))
        wpool = ctx.enter_context(tc.tile_pool(name="w", bufs=3))
        apool = ctx.enter_context(tc.tile_pool(name="a", bufs=1))
        tpool = ctx.enter_context(tc.tile_pool(name="t", bufs=2))
        opool = ctx.enter_context(tc.tile_pool(name="o", bufs=4))
        wdpool = ctx.enter_context(tc.tile_pool(name="wd", bufs=4))

        # PE clock-gate warmup: HAM starts at 1.2 GHz and un-throttles only
        # after ~3.4us of sustained activity. Real matmuls can't start until
        # the first weights + x tiles land from HBM (~13us). Dummy bf16
        # matmuls on memset scratch need no DMA, so they run right at launch
        # and the real stream begins at 2.4 GHz. The count is sized to end
        # roughly when the first real matmul's operands land (PE queue is
        # FIFO, so an oversized warmup delays the real stream).
        with (
            tc.tile_pool(name="warm", bufs=1) as wmpool,
            tc.tile_pool(name="warmps", bufs=1, space="PSUM") as wmpspool,
        ):
            wt = wmpool.tile([128, 512], bf16, name="warm_in")
            nc.vector.memset(wt[:], 0.0)
            wps = wmpspool.tile([128, 512], f32, name="warm_ps")
            for r in range(NWARM):
                nc.tensor.matmul(wps[:], wt[:, 0:128], wt[:], start=True, stop=True)

        w_tiles = {}

        def alloc_w(i):
            wg_gs = [
                wpool.tile([128, G * 128], bf16, tag=f"wg{g}", name=f"wg{i}_{g}")
                for g in range(GU)
            ]
            wu_gs = [
                wpool.tile([128, G * 128], bf16, tag=f"wu{g}", name=f"wu{i}_{g}")
                for g in range(GU)
            ]
            w_tiles[i] = (wg_gs, wu_gs)
            return wg_gs, wu_gs

        def issue_w(i):
            wg_gs, wu_gs = alloc_w(i)
            for g in range(GU):
                nc.sync.dma_start(wg_gs[g][:], Wg_d[i, g])
            for g in range(GU):
                nc.sync.dma_start(wu_gs[g][:], Wu_d[i, g])

        xts = {}

        def alloc_x(n, j):
            t = xpool.tile([128, 2 * N], bf16, name=f"x{n}_{j}")
            xts[(n, j)] = t
            return t

        def issue_x(n, js):
            for j in js:
                nc.sync.dma_start(alloc_x(n, j)[:], xP[n, j])

        # Ramp emission order (sync ring is FIFO, drained in issue order):
        # interleave the i=0 weight tiles with the n=0 x tiles in the order
        # group (i=0, n=0) consumes them, so the real matmul stream starts
        # DMA-paced as soon as the first ~0.6MB lands instead of waiting
        # for the whole ~3MB first-group working set.
        wg0, wu0 = alloc_w(0)
        half = -(-KH // 2) // GU  # x-pair tiles consumed per weight group
        for g in range(GU):
            nc.sync.dma_start(wg0[g][:], Wg_d[0, g])
            nc.sync.dma_start(wu0[g][:], Wu_d[0, g])
            issue_x(0, range(g * half, min((g + 1) * half, KH // 2)))
        if KI > 1:
            issue_w(1)
        for n in range(1, NT):
            issue_x(n, range(KH // 2))
        if KI > 2:
            issue_w(2)

        aT = apool.tile([128, KI, C], bf16, name="aT")

        wd_tiles = {}

        def issue_wd(h):
            wd_gs = []
            for g in range(GD):
                wdt = wdpool.tile([128, G * 128], bf16, tag=f"wd{g}", name=f"wd{h}_{g}")
                nc.sync.dma_start(wdt[:], Wd_d[h, g])
                wd_gs.append(wdt)
            wd_tiles[h] = wd_gs

        # Both PSUM pools stay open for the whole kernel (2*2 + 4 = 8
        # banks): closing gu before opening d would insert a drain barrier
        # (~1.5-3.5us of PE idle at the up->down transition).
        with (
            tc.tile_pool(name="gu", bufs=2, space="PSUM") as gupool,
            tc.tile_pool(name="d", bufs=4, space="PSUM") as dpool,
        ):
            # Up phase: i-outer so each weight tile is loaded exactly once.
            for i in range(KI):
                if i + 3 < KI and i + 3 not in w_tiles:
                    issue_w(i + 3)
                # prefetch the first down-phase weights near the end
                if i >= KI - 3 and (h := i - (KI - 3)) < 3:
                    issue_wd(h)
                if i not in w_tiles:
                    issue_w(i)
                wg_gs, wu_gs = w_tiles.pop(i)
                for n in range(NT):
                    g_ps = gupool.tile([128, N], f32, tag="g", name=f"g{i}_{n}")
                    u_ps = gupool.tile([128, N], f32, tag="u", name=f"u{i}_{n}")
                    nsl = slice(n * N, (n + 1) * N)
                    for k in range(KH):
                        ksl = slice((k % G) * 128, (k % G + 1) * 128)
                        xs = xts[(n, k // 2)][:, (k % 2) * N : (k % 2 + 1) * N]
                        nc.tensor.matmul(
                            g_ps[:],
                            wg_gs[k // G][:, ksl],
                            xs,
                            start=(k == 0),
                            stop=(k == KH - 1),
                        )
                        nc.tensor.matmul(
                            u_ps[:],
                            wu_gs[k // G][:, ksl],
                            xs,
                            start=(k == 0),
                            stop=(k == KH - 1),
                        )
                    gel = tpool.tile([128, N], f32, tag="gel", name=f"gel{i}_{n}")
                    nc.scalar.activation(gel[:], g_ps[:], GELU)
                    nc.vector.tensor_mul(aT[:, i, nsl], gel[:], u_ps[:])

            # Down phase: d^T[h] = sum_ki Wd[ki,h]^T @ aT[ki]. Output DMAs
            # go on the Act HWDGE ring (nc.scalar) so they never queue
            # behind input prefetches on the sync ring.
            for h in range(KH):
                if h + 3 < KH and h + 3 not in wd_tiles:
                    issue_wd(h + 3)
                if h not in wd_tiles:
                    issue_wd(h)
                wd_gs = wd_tiles.pop(h)
                for n in range(NT):
                    d_ps = dpool.tile([128, N], f32, tag="d", name=f"d{h}_{n}")
                    nsl = slice(n * N, (n + 1) * N)
                    for ki in range(KI):
                        lw = wd_gs[ki // G][:, (ki % G) * 128 : (ki % G + 1) * 128]
                        nc.tensor.matmul(
                            d_ps[:],
                            lw,
                            aT[:, ki, nsl],
                            start=(ki == 0),
                            stop=(ki == KI - 1),
                        )
                    o = opool.tile([128, N], f32, tag="o", name=f"o{h}_{n}")
                    # Alternate output DMAs across the two HWDGE rings
                    # (receipt latencies overlap; neither ring backs up).
                    # Split the final tiles so the tail isn't gated on one
                    # long copy + transfer + HBM write receipt.
                    nch = 4 if h == KH - 1 else 1
                    ch = -(-N // nch)
                    ch += ch % 2
                    for c in range(0, N, ch):
                        w = min(ch, N - c)
                        nc.vector.tensor_copy(o[:, c : c + w], d_ps[:, c : c + w])
                        eng = nc.scalar if (h * NT + n + c // ch) % 2 == 0 else nc.sync
                        eng.dma_start(
                            dT_p[:, h, n * N + c : n * N + c + w],
                            o[:, c : c + w],
                        )

    nc.compile()
    return nc


def _get_program(C, NT, N):
    key = (C, NT, N, NWARM)
    if key not in _PROGRAM_CACHE:
        _PROGRAM_CACHE[key] = _build_program(C, NT, N)
    return _PROGRAM_CACHE[key]


def _ensure_ntff_hook():
    """Register the axon NTFF profile hook if the image's antenv lacks
    axon_hooks (see trn_agent_boot.trn_boot). Only needed when TRACE."""
    import types

    try:
        from antenv.axon_hooks import get_axon_ntff_profile_hook  # noqa: F401

        return
    except ImportError:
        pass
    import antenv
    from trn_agent_boot.trn_boot import _ntff_profile_via_ctypes

    hook = _ntff_profile_via_ctypes("/opt/axon/libaxon_pjrt.so")
    mod = types.ModuleType("antenv.axon_hooks")
    state = {"hook": hook}
    mod.set_axon_ntff_profile_hook = lambda h: state.__setitem__("hook", h)
    mod.get_axon_ntff_profile_hook = lambda: state["hook"]
    sys.modules["antenv.axon_hooks"] = mod
    antenv.axon_hooks = mod


def kernel(x, Wg, Wu, Wd, selected_experts, routing_weights):
    global LAST_RESULT
    import ml_dtypes
    from concourse.bass_utils import run_bass_kernel_spmd

    if TRACE:
        _ensure_ntff_hook()

    bf16 = ml_dtypes.bfloat16
    x = np.asarray(x, dtype=np.float32)
    Wg = np.asarray(Wg, dtype=np.float32)
    Wu = np.asarray(Wu, dtype=np.float32)
    Wd = np.asarray(Wd, dtype=np.float32)
    selected_experts = np.asarray(selected_experts)
    routing_weights = np.asarray(routing_weights, dtype=np.float32)

    # Host-side dispatch: per expert, the (deduplicated) token list and
    # summed routing weights.
    idx_list, w_list = [], []
    for e in range(E):
        m = selected_experts == e  # [T, K]
        idx = np.nonzero(m.any(axis=1))[0]
        w = (routing_weights * m).sum(axis=1)[idx]
        idx_list.append(idx)
        w_list.append(w.astype(np.float32))

    max_count = max(len(idx) for idx in idx_list)
    C, NT, N = _pick_config(max_count)

    nc = _get_program(C, NT, N)

    in_maps = []
    for e in range(E):
        idx = idx_list[e]
        xT = np.zeros((H, C), dtype=bf16)
        xT[:, : len(idx)] = x[idx].T.astype(bf16)
        # pack k-tile pairs: [n, j, p, kk*N+c] = xT[(2j+kk)*128+p, n*N+c]
        xPk = (
            xT.reshape(KH // 2, 2, 128, NT, N)
            .transpose(3, 0, 2, 1, 4)
            .reshape(NT, KH // 2, 128, 2 * N)
        )
        in_maps.append(
            {
                "xP": np.ascontiguousarray(xPk),
                "Wg": _tile_w_up(Wg[e].astype(bf16)),
                "Wu": _tile_w_up(Wu[e].astype(bf16)),
                "Wd": _tile_w_down(Wd[e].astype(bf16)),
            }
        )

    res = run_bass_kernel_spmd(
        nc,
        in_maps,
        list(range(NCORES)),
        trace=TRACE,
        trace_cores=TRACE_CORES if TRACE else None,
    )
    LAST_RESULT = res

    out = np.zeros((T, H), dtype=np.float32)
    for e in range(E):
        idx = idx_list[e]
        dTe = res.results[e]["dT"]  # [H, C] fp32
        out[idx] += w_list[e][:, None] * dTe[:, : len(idx)].T
    return out


# revision 6
# speedup vs baseline: 1.0275x; 1.0275x over previous
"""Gemma4 MoE feed-forward on 8 Trainium2 NeuronCores.

Strategy: expert-parallel. E == n_cores == 8, so core e owns expert e's
weights (Wg[e], Wu[e], Wd[e]) and receives exactly the tokens routed to
expert e (gathered + transposed + padded on the host). Each core runs a
dense gated-FFN over its token batch:

    dT = Wd^T @ (gelu_tanh(Wg^T x^T) * (Wu^T x^T))        (all [*, C] layouts)

The host then scatter-adds routing_weight * dT^T back into the full
[T, H] output. Tokens that select the same expert in both slots are
deduplicated on the host (weights summed).

All matmul operands are bf16 (rel err ~4e-3 vs the 2e-2 gate): bf16
streams the moving operand ~20ns/MM faster than fp32r and halves HBM
traffic. The up phase runs i-outer / n-inner so each Wg/Wu tile is
DMA'd exactly once (n-outer re-streamed all up weights per token block
and saturated the 358 GB/s per-core HBM budget). DMAs are consolidated
into few large transfers: each dma_start costs ~650ns of serial
sequencer issue time, so the startup ramp is paced by issue count as
much as by bytes.
"""

import os
import sys

import numpy as np

for _p in ("/opt/trn_rl_repo", "/root/.axon_site/_ro/trn_rl_repo"):
    if os.path.isdir(_p) and _p not in sys.path:
        sys.path.append(_p)

T, H, I, E, K = 4096, 2048, 1024, 8, 2
NCORES = 8
KH = H // 128  # 16 k-tiles over the hidden dim
KI = I // 128  # 8 k-tiles over the intermediate dim

_PROGRAM_CACHE = {}
LAST_RESULT = None  # BassKernelResults of the most recent run (for test.py)
TRACE = False  # test.py sets this to capture an NTFF profile
TRACE_CORES = [0]

NWARM = int(os.environ.get("MOE_NWARM", "26"))
OTAIL = 128  # final down-group split width (shortens the last write chain)


def _tile_w_up(W):
    """[H, I] -> [KI, 128, KH*128] with [i, p, k*128+c] = W[k*128+p, i*128+c]:
    the i-tile's whole lhsT set is one DMA, 4KB contiguous per partition."""
    return np.ascontiguousarray(
        W.reshape(KH, 128, KI, 128).transpose(2, 1, 0, 3).reshape(KI, 128, KH * 128)
    )


def _tile_w_down(W):
    """[I, H] -> [KH, 128, KI*128], same scheme (contraction over I)."""
    return np.ascontiguousarray(
        W.reshape(KI, 128, KH, 128).transpose(2, 1, 0, 3).reshape(KH, 128, KI * 128)
    )


def _pick_config(max_count):
    """Minimal uniform token-block config: NT blocks of even width N with
    NT*N >= max_count, N <= 512 (PSUM bank limit) and N >= 256 (so the
    ~100ns LDWEIGHTS fully hides under the moving-operand stream)."""
    mc = max(max_count, 256)
    nt = -(-mc // 512)
    n = -(-mc // nt)
    n += n % 2
    return (nt * n, nt, n)  # (C, NT, N)


def _build_program(C, NT, N):
    import concourse.tile as tile
    from concourse import bacc, mybir
    from contextlib import ExitStack

    f32 = mybir.dt.float32
    bf16 = mybir.dt.bfloat16

    nc = bacc.Bacc("TRN2", target_bir_lowering=False, debug=False)

    # x arrives host-packed: [n, p, k*N+c] = x^T[k*128+p, n*N+c], so each
    # n-block is one contiguous KH*N*2 = ~16KB run per partition.
    xP = nc.dram_tensor("xP", [NT, 128, KH * N], bf16, kind="ExternalInput").ap()
    Wg_d = nc.dram_tensor("Wg", [KI, 128, KH * 128], bf16, kind="ExternalInput").ap()
    Wu_d = nc.dram_tensor("Wu", [KI, 128, KH * 128], bf16, kind="ExternalInput").ap()
    Wd_d = nc.dram_tensor("Wd", [KH, 128, KI * 128], bf16, kind="ExternalInput").ap()
    dT = nc.dram_tensor("dT", [H, C], f32, kind="ExternalOutput").ap()

    # Partition-major view: row a*128+p -> partition p, free index a.
    dT_p = dT.rearrange("(a p) c -> p a c", p=128)  # [128, KH, C]

    GELU = mybir.ActivationFunctionType.Gelu_apprx_tanh

    with tile.TileContext(nc) as tc, ExitStack() as ctx:
        xpool = ctx.enter_context(tc.tile_pool(name="x", bufs=1))
        wpool = ctx.enter_context(tc.tile_pool(name="w", bufs=3))
        apool = ctx.enter_context(tc.tile_pool(name="a", bufs=1))
        tpool = ctx.enter_context(tc.tile_pool(name="t", bufs=2))
        opool = ctx.enter_context(tc.tile_pool(name="o", bufs=4))
        wdpool = ctx.enter_context(tc.tile_pool(name="wd", bufs=4))

        # PE clock-gate warmup: HAM starts at 1.2 GHz and un-throttles only
        # after ~3.4us of sustained activity. Real matmuls can't start until
        # the first weights + x block land from HBM (~16us: ~8us framework
        # preamble before the first DMA byte moves, then ~3MB at the HBM
        # rate). Dummy bf16 matmuls on memset scratch need no DMA, so they
        # run right at launch and the real stream begins at 2.4 GHz. The
        # count is sized to end just as the first real matmul's operands
        # land: shorter leaves the real stream DMA-paced with PE gaps (HAM
        # re-throttles, costing ~5us); longer delays the stream (PE queue
        # is FIFO).
        with (
            tc.tile_pool(name="warm", bufs=1) as wmpool,
            tc.tile_pool(name="warmps", bufs=1, space="PSUM") as wmpspool,
        ):
            wt = wmpool.tile([128, 512], bf16, name="warm_in")
            nc.vector.memset(wt[:], 0.0)
            wps = wmpspool.tile([128, 512], f32, name="warm_ps")
            for r in range(NWARM):
                nc.tensor.matmul(wps[:], wt[:, 0:128], wt[:], start=True, stop=True)

        w_tiles = {}

        def issue_w(i):
            wgt = wpool.tile([128, KH * 128], bf16, tag="wg", name=f"wg{i}")
            wut = wpool.tile([128, KH * 128], bf16, tag="wu", name=f"wu{i}")
            nc.sync.dma_start(wgt[:], Wg_d[i])
            nc.sync.dma_start(wut[:], Wu_d[i])
            w_tiles[i] = (wgt, wut)

        xts = {}

        def issue_x(n):
            t = xpool.tile([128, KH * N], bf16, name=f"x{n}")
            hw = (KH // 2) * N
            nc.sync.dma_start(t[:, 0:hw], xP[n, :, 0:hw])
            nc.sync.dma_start(t[:, hw:], xP[n, :, hw:])
            xts[n] = t

        # Ramp emission order (sync ring is FIFO, drained in issue order):
        # i=0 weights, then ALL x blocks, then i=1,2 weights. The first
        # (i,n) groups consume x faster than HBM supplies it, so x must
        # outrank the i>=1 weight prefetches or the stream stalls at
        # (i=0, n=1).
        issue_w(0)
        for n in range(NT):
            issue_x(n)
        for i in (1, 2):
            if i < KI:
                issue_w(i)

        aT = apool.tile([128, KI, C], bf16, name="aT")

        wd_tiles = {}

        def issue_wd(h):
            wdt = wdpool.tile([128, KI * 128], bf16, tag="wd", name=f"wd{h}")
            nc.sync.dma_start(wdt[:], Wd_d[h])
            wd_tiles[h] = wdt

        # Both PSUM pools stay open for the whole kernel (2*2 + 4 = 8
        # banks): closing gu before opening d would insert a drain barrier
        # (~1.5-3.5us of PE idle at the up->down transition).
        with (
            tc.tile_pool(name="gu", bufs=2, space="PSUM") as gupool,
            tc.tile_pool(name="d", bufs=4, space="PSUM") as dpool,
        ):
            # Up phase: i-outer so each weight tile is loaded exactly once.
            for i in range(KI):
                if i + 3 < KI and i + 3 not in w_tiles:
                    issue_w(i + 3)
                # prefetch the first down-phase weights near the end
                if i >= KI - 3 and (h := i - (KI - 3)) < 3:
                    issue_wd(h)
                if i not in w_tiles:
                    issue_w(i)
                wgt, wut = w_tiles.pop(i)
                for n in range(NT):
                    g_ps = gupool.tile([128, N], f32, tag="g", name=f"g{i}_{n}")
                    u_ps = gupool.tile([128, N], f32, tag="u", name=f"u{i}_{n}")
                    nsl = slice(n * N, (n + 1) * N)
                    for k in range(KH):
                        ksl = slice(k * 128, (k + 1) * 128)
                        xs = xts[n][:, k * N : (k + 1) * N]
                        nc.tensor.matmul(
                            g_ps[:], wgt[:, ksl], xs,
                            start=(k == 0), stop=(k == KH - 1),
                        )
                        nc.tensor.matmul(
                            u_ps[:], wut[:, ksl], xs,
                            start=(k == 0), stop=(k == KH - 1),
                        )
                    gel = tpool.tile([128, N], f32, tag="gel", name=f"gel{i}_{n}")
                    nc.scalar.activation(gel[:], g_ps[:], GELU)
                    nc.vector.tensor_mul(aT[:, i, nsl], gel[:], u_ps[:])

            # Down phase: d^T[h] = sum_ki Wd[ki,h]^T @ aT[ki]. Output DMAs
            # alternate between the two HWDGE rings (nc.scalar / nc.sync)
            # so HBM write receipts overlap and neither ring backs up. The
            # very last group is split into a wide piece and an OTAIL-wide
            # piece so the final copy+DMA+receipt chain is short.
            for h in range(KH):
                if h + 3 < KH and h + 3 not in wd_tiles:
                    issue_wd(h + 3)
                if h not in wd_tiles:
                    issue_wd(h)
                wdt = wd_tiles.pop(h)
                for n in range(NT):
                    last = h == KH - 1 and n == NT - 1
                    splits = (
                        [(0, N - OTAIL), (N - OTAIL, N)] if last else [(0, N)]
                    )
                    for si, (c0, c1) in enumerate(splits):
                        d_ps = dpool.tile(
                            [128, c1 - c0], f32, tag="d", name=f"d{h}_{n}_{si}"
                        )
                        for ki in range(KI):
                            nc.tensor.matmul(
                                d_ps[:],
                                wdt[:, ki * 128 : (ki + 1) * 128],
                                aT[:, ki, n * N + c0 : n * N + c1],
                                start=(ki == 0),
                                stop=(ki == KI - 1),
                            )
                        o = opool.tile(
                            [128, c1 - c0], f32, tag="o", name=f"o{h}_{n}_{si}"
                        )
                        nc.vector.tensor_copy(o[:], d_ps[:])
                        eng = nc.scalar if (h * NT + n + si) % 2 == 0 else nc.sync
                        eng.dma_start(
                            dT_p[:, h, n * N + c0 : n * N + c1], o[:]
                        )

    nc.compile()
    return nc


def _get_program(C, NT, N):
    key = (C, NT, N, NWARM)
    if key not in _PROGRAM_CACHE:
        _PROGRAM_CACHE[key] = _build_program(C, NT, N)
    return _PROGRAM_CACHE[key]


def _ensure_ntff_hook():
    """Register the axon NTFF profile hook if the image's antenv lacks
    axon_hooks (see trn_agent_boot.trn_boot). Only needed when TRACE."""
    import types

    try:
        from antenv.axon_hooks import get_axon_ntff_profile_hook  # noqa: F401

        return
    except ImportError:
        pass
    import antenv
    from trn_agent_boot.trn_boot import _ntff_profile_via_ctypes

    hook = _ntff_profile_via_ctypes("/opt/axon/libaxon_pjrt.so")
    mod = types.ModuleType("antenv.axon_hooks")
    state = {"hook": hook}
    mod.set_axon_ntff_profile_hook = lambda h: state.__setitem__("hook", h)
    mod.get_axon_ntff_profile_hook = lambda: state["hook"]
    sys.modules["antenv.axon_hooks"] = mod
    antenv.axon_hooks = mod


def kernel(x, Wg, Wu, Wd, selected_experts, routing_weights):
    global LAST_RESULT
    import ml_dtypes
    from concourse.bass_utils import run_bass_kernel_spmd

    if TRACE:
        _ensure_ntff_hook()

    bf16 = ml_dtypes.bfloat16
    x = np.asarray(x, dtype=np.float32)
    Wg = np.asarray(Wg, dtype=np.float32)
    Wu = np.asarray(Wu, dtype=np.float32)
    Wd = np.asarray(Wd, dtype=np.float32)
    selected_experts = np.asarray(selected_experts)
    routing_weights = np.asarray(routing_weights, dtype=np.float32)

    # Host-side dispatch: per expert, the (deduplicated) token list and
    # summed routing weights.
    idx_list, w_list = [], []
    for e in range(E):
        m = selected_experts == e  # [T, K]
        idx = np.nonzero(m.any(axis=1))[0]
        w = (routing_weights * m).sum(axis=1)[idx]
        idx_list.append(idx)
        w_list.append(w.astype(np.float32))

    max_count = max(len(idx) for idx in idx_list)
    C, NT, N = _pick_config(max_count)

    nc = _get_program(C, NT, N)

    in_maps = []
    for e in range(E):
        idx = idx_list[e]
        xT = np.zeros((H, C), dtype=bf16)
        xT[:, : len(idx)] = x[idx].T.astype(bf16)
        # pack [n, p, k*N+c] = xT[k*128+p, n*N+c]
        xPk = (
            xT.reshape(KH, 128, NT, N).transpose(2, 1, 0, 3).reshape(NT, 128, KH * N)
        )
        in_maps.append(
            {
                "xP": np.ascontiguousarray(xPk),
                "Wg": _tile_w_up(Wg[e].astype(bf16)),
                "Wu": _tile_w_up(Wu[e].astype(bf16)),
                "Wd": _tile_w_down(Wd[e].astype(bf16)),
            }
        )

    res = run_bass_kernel_spmd(
        nc,
        in_maps,
        list(range(NCORES)),
        trace=TRACE,
        trace_cores=TRACE_CORES if TRACE else None,
    )
    LAST_RESULT = res

    out = np.zeros((T, H), dtype=np.float32)
    for e in range(E):
        idx = idx_list[e]
        dTe = res.results[e]["dT"]  # [H, C] fp32
        out[idx] += w_list[e][:, None] * dTe[:, : len(idx)].T
    return out
